# revision 24
# baseline (speedup 1.0000x reference)
"""Euclidean-distance attention Trainium2 Bass kernel.

Problem: B=8, Sq=Sk=2048, D=512, fp32.
  dist2[q,k]  = ||Q_q||^2 + ||K_k||^2 - 2 Q_q.K_k
  W = softmax(-dist2 / temp, axis=k);  O = W @ V
Outputs: (attended [B,Sq,D], weights [B,Sq,Sk]).

Sharding: data-parallel over batch -> 1 batch element per NeuronCore (8 cores).

Per-core algorithm (softmax is invariant to the per-row constant ||Q_q||^2,
so it is dropped; mathematically identical result):
  sim[q,k] = (2 Q.K^T - ||K||^2) / temp
  W = softmax(sim, k);  O = W @ V

Engine mapping per 128-row q tile:
  PE : QK^T in float32r (1 cy/row), W^T transposes in bf16 (1 cy/row),
       W@V in bf16 (1 cy/row)
  DVE: (S_psum - ksq_bcast) subtract, row max, PV output row scale
  ACT: exp with per-partition bias = -rowmax and fused row-sum (accum_out),
       half of the PSUM->SBUF W^T copies, prolog transpose copies
  GPS: ksq broadcast, weight normalization (1/l row scale)
"""

import numpy as np
from contextlib import ExitStack

import concourse.bass as bass
import concourse.tile as tile
from concourse import bacc, mybir
from concourse.bass import ts
from concourse.bass_utils import run_bass_kernel_spmd
from concourse.masks import make_identity

F32 = mybir.dt.float32
F32R = mybir.dt.float32r
BF16 = mybir.dt.bfloat16
AX = mybir.AxisListType
AF = mybir.ActivationFunctionType

S_FULL = 2048
D_FULL = 512
N_CORES = 8


def build_kernel(S=S_FULL, D=D_FULL, use_f32r=True, repeat=1,
                 bufs_big=3, wnorm_first=False, wt_on_dve_frac=0,
                 ablate=(), nchunk=512, s_bufs=None, t_bufs=2,
                 exp_chunk=None, wt_chunk=1024, sub_inplace=False,
                 wdma_act=False, pipeline_emit=False, wt_dma_tr=False):
    """Build the single-core program (one batch element)."""
    P = 128
    NT = S // P          # 128-row tiles along Sq / Sk
    DC = D // P          # 128-deep contraction chunks of d
    nchunk = min(nchunk, S)
    wt_chunk = min(wt_chunk, S)
    NN = S // nchunk     # softmax chunks of the k axis
    NMM = nchunk // 512  # matmuls per chunk (N=512 each)
    if exp_chunk is None:
        exp_chunk = nchunk
    NE = S // exp_chunk  # exp chunks
    MMDT = F32R if use_f32r else F32

    nc = bacc.Bacc("TRN2", target_bir_lowering=False, debug=False)
    q_d = nc.dram_tensor("query", [S, D], F32, kind="ExternalInput").ap()
    k_d = nc.dram_tensor("key", [S, D], F32, kind="ExternalInput").ap()
    v_d = nc.dram_tensor("value", [S, D], F32, kind="ExternalInput").ap()
    t_d = nc.dram_tensor("temperature", [1, 1], F32, kind="ExternalInput").ap()
    w_d = nc.dram_tensor("weights", [S, S], F32, kind="ExternalOutput").ap()
    o_d = nc.dram_tensor("attended", [S, D], F32, kind="ExternalOutput").ap()

    with tile.TileContext(nc) as tc, ExitStack() as ctx:
        const = ctx.enter_context(tc.tile_pool(name="const", bufs=1))
        vt_p = ctx.enter_context(tc.tile_pool(name="vt", bufs=S // P))
        kt_p = ctx.enter_context(tc.tile_pool(name="kt", bufs=S // 512))
        qt_p = ctx.enter_context(tc.tile_pool(name="qt", bufs=S // P))
        ld_p = ctx.enter_context(tc.tile_pool(name="ld", bufs=3))
        sq_p = ctx.enter_context(tc.tile_pool(name="sq", bufs=2))
        st_p = ctx.enter_context(tc.tile_pool(name="st", bufs=bufs_big + 1))
        sim_p = ctx.enter_context(tc.tile_pool(name="sim", bufs=bufs_big))
        e_p = ctx.enter_context(tc.tile_pool(name="e", bufs=bufs_big))
        w_p = ctx.enter_context(tc.tile_pool(name="w", bufs=2))
        wt_p = ctx.enter_context(tc.tile_pool(name="wtt", bufs=bufs_big))
        o_p = ctx.enter_context(tc.tile_pool(name="o", bufs=2))
        dram_p = ctx.enter_context(tc.tile_pool(name="dr", bufs=1, space="DRAM"))
        if s_bufs is None:
            s_bufs = {512: 5, 1024: 3, 2048: 1}[nchunk]
        s_ps = ctx.enter_context(tc.tile_pool(name="sps", bufs=s_bufs, space="PSUM"))
        t_ps = ctx.enter_context(tc.tile_pool(name="tps", bufs=t_bufs, space="PSUM"))
        o_ps = ctx.enter_context(tc.tile_pool(name="ops", bufs=1, space="PSUM"))

        import contextlib
        loop_cm = tc.For_i(0, repeat, 1) if repeat > 1 else contextlib.nullcontext()
        with loop_cm:
            # ---- constants / scalars ----
            ident = const.tile([P, P], F32, tag="ident")
            make_identity(nc, ident[:])
            ident_bf = const.tile([P, P], BF16, tag="ident_bf")
            nc.vector.tensor_copy(ident_bf[:], ident[:])

            t_sb = const.tile([1, 1], F32, tag="t_sb")
            inv_t = const.tile([1, 1], F32, tag="inv_t")
            inv_t_b = const.tile([P, 1], F32, tag="inv_t_b")
            two_inv_t = const.tile([P, 1], F32, tag="two_inv_t")
            nc.sync.dma_start(t_sb[:], t_d[:])
            nc.vector.reciprocal(inv_t[:], t_sb[:])
            nc.gpsimd.partition_broadcast(inv_t_b[:], inv_t[0:1, 0:1])
            nc.scalar.mul(two_inv_t[:], inv_t_b[:], 2.0)

            # ---- K: load, ||K||^2, transpose into per-512-block KT tiles ----
            # kt_nb[nb] holds [c][k_local] for k in [nb*512,(nb+1)*512)
            kt_nb = [kt_p.tile([P, DC * 512], MMDT, tag="kt_nb", name=f"ktnb{b}") for b in range(S // 512)]
            ksq = const.tile([P, NT], F32, tag="ksq")
            for t in range(NT):
                kn = ld_p.tile([P, D], F32, tag="nat")
                nc.sync.dma_start(kn[:], k_d[ts(t, P), :])
                sq = sq_p.tile([P, D], F32, tag="sqs")
                nc.scalar.activation(
                    sq[:], kn[:], AF.Square, accum_out=ksq[:, t : t + 1]
                )
                ps = s_ps.tile([P, 512], F32, tag="s")
                for c in range(DC):
                    nc.tensor.transpose(ps[:, ts(c, P)], kn[:, ts(c, P)], ident[:])
                nb, tl = t // 4, t % 4
                out_ap = kt_nb[nb][:].rearrange("p (c k) -> p c k", k=512)[
                    :, :, ts(tl, P)
                ]
                nc.scalar.activation(
                    out_ap,
                    ps[:].rearrange("p (c j) -> p c j", j=P),
                    AF.Copy,
                    scale=two_inv_t[:, 0:1],
                )

            # ---- Q: load + transpose into per-q-tile QT tiles ----
            qt_i = []
            for i in range(NT):
                qn = ld_p.tile([P, D], F32, tag="nat")
                nc.sync.dma_start(qn[:], q_d[ts(i, P), :])
                ps = s_ps.tile([P, 512], F32, tag="s")
                for c in range(DC):
                    nc.tensor.transpose(ps[:, ts(c, P)], qn[:, ts(c, P)], ident[:])
                qt = qt_p.tile([P, D], MMDT, tag="qt_i", name=f"qti{i}")
                nc.scalar.copy(qt[:], ps[:])
                qt_i.append(qt)

            # ---- V: per-tile bf16 (PV matmul rhs) ----
            v_j = []
            for t in range(NT):
                vn = ld_p.tile([P, D], F32, tag="nat")
                nc.sync.dma_start(vn[:], v_d[ts(t, P), :])
                vt = vt_p.tile([P, D], BF16, tag="v_j", name=f"vj{t}")
                nc.scalar.copy(vt[:], vn[:])
                v_j.append(vt)

            # ---- ||K||^2/temp broadcast to [128, S] (full fp32) ----
            ksq_n = const.tile([P, NT], F32, tag="ksq_n")
            nc.vector.tensor_scalar_mul(ksq_n[:], ksq[:], inv_t_b[:, 0:1])
            psk = t_ps.tile([NT, P], F32, tag="tps")
            nc.tensor.transpose(psk[:], ksq_n[:], ident[:])
            sbk = const.tile([NT, P], F32, tag="ksqT")
            nc.vector.tensor_copy(sbk[:], psk[:])
            scr = dram_p.tile([1, S], F32, tag="scr")
            nc.sync.dma_start(scr[:].rearrange("a (t p) -> (a t) p", p=P), sbk[:])
            ksq_row = const.tile([1, S], F32, tag="ksq_row")
            nc.sync.dma_start(ksq_row[:], scr[:])
            ksq_bc = const.tile([P, S], F32, tag="ksq_bc")
            nc.gpsimd.partition_broadcast(ksq_bc[:], ksq_row[:])

            # ---- main loop over q tiles (emission software-pipelined:
            # tile i+1's matmuls are emitted before tile i's tail so the
            # scheduler prioritizes keeping the PE fed) ----
            def emit_head(i):
                st = {}
                st["i"] = i
                st["mx"] = st_p.tile([P, NN], F32, tag="mx", name=f"mx{i}")
                st["sim"] = sim_p.tile([P, S], F32, tag="sim", name=f"sim{i}")
                for n in range(NN):
                    ps = s_ps.tile([P, nchunk], F32, tag="s", name=f"s{i}_{n}")
                    for m in range(NMM):
                        for c in range(DC):
                            nc.tensor.matmul(
                                ps[:, ts(m, 512)],
                                qt_i[i][:, ts(c, P)],
                                kt_nb[n * NMM + m][:, ts(c, 512)],
                                start=(c == 0),
                                stop=(c == DC - 1),
                            )
                    nc.vector.tensor_sub(
                        st["sim"][:, ts(n, nchunk)], ps[:], ksq_bc[:, ts(n, nchunk)]
                    )
                    nc.vector.reduce_max(
                        st["mx"][:, n : n + 1], st["sim"][:, ts(n, nchunk)], axis=AX.X
                    )
                return st

            def emit_tail(st):
                i = st["i"]
                mx, sim_t = st["mx"], st["sim"]
                l4 = st_p.tile([P, NN], F32, tag="l4", name=f"l4{i}")
                negm = st_p.tile([P, 1], F32, tag="negm", name=f"negm{i}")
                lsum = st_p.tile([P, 1], F32, tag="lsum", name=f"lsum{i}")
                linv = st_p.tile([P, 1], F32, tag="linv", name=f"linv{i}")
                nc.vector.reduce_max(negm[:], mx[:], axis=AX.X, negate=True)

                e_t = e_p.tile([P, S], BF16, tag="e", name=f"e{i}")
                for n in range(NE):
                    nc.scalar.activation(
                        e_t[:, ts(n, exp_chunk)],
                        sim_t[:, ts(n, exp_chunk)],
                        AF.Exp,
                        bias=negm[:, 0:1],
                        accum_out=l4[:, n : n + 1],
                    )
                if NE > 1:
                    nc.vector.reduce_sum(lsum[:], l4[:, 0:NE], axis=AX.X)
                else:
                    nc.vector.tensor_copy(lsum[:], l4[:, 0:1])
                nc.vector.reciprocal(linv[:], lsum[:])

                # W^T tiles in bf16 (transpose unnormalized exp)
                wt_t = wt_p.tile([P, S], BF16, tag="wt", name=f"wt{i}")
                if wt_dma_tr:
                    # hardware DMA transpose (2-byte dtype): out[p, t, j] =
                    # e[j, t*128+p], exactly the W^T block layout
                    nc.scalar.dma_start_transpose(
                        wt_t[:].rearrange("p (t j) -> p t j", j=P), e_t[:]
                    )
                else:
                    for b in range(S // wt_chunk):
                        wps = t_ps.tile([P, wt_chunk], BF16, tag="tps", name=f"tps{i}_{b}")
                        for j2 in range(wt_chunk // P):
                            j = b * (wt_chunk // P) + j2
                            nc.tensor.transpose(
                                wps[:, ts(j2, P)], e_t[:, ts(j, P)], ident_bf[:]
                            )
                        nc.scalar.copy(wt_t[:, ts(b, wt_chunk)], wps[:])

                # O = (e^T)^T @ V then row-scale by 1/l
                opsum = o_ps.tile([P, D], F32, tag="o_ps", name=f"ops{i}")
                for j in range(NT):
                    nc.tensor.matmul(
                        opsum[:],
                        wt_t[:, ts(j, P)],
                        v_j[j][:],
                        start=(j == 0),
                        stop=(j == NT - 1),
                    )
                o_t = o_p.tile([P, D], F32, tag="o", name=f"o{i}")
                nc.vector.tensor_scalar_mul(o_t[:], opsum[:], linv[:, 0:1])
                nc.sync.dma_start(o_d[ts(i, P), :], o_t[:])

                # normalized fp32 weights -> DRAM
                w_t = w_p.tile([P, S], F32, tag="w", name=f"w{i}")
                nc.vector.tensor_scalar_mul(w_t[:], e_t[:], linv[:, 0:1])
                eng = nc.scalar if wdma_act else nc.sync
                eng.dma_start(w_d[ts(i, P), :], w_t[:])

            if pipeline_emit:
                prev = emit_head(0)
                for i in range(1, NT):
                    cur = emit_head(i)
                    emit_tail(prev)
                    prev = cur
                emit_tail(prev)
            else:
                for i in range(NT):
                    emit_tail(emit_head(i))

    nc.compile()
    return nc


_NC_CACHE = {}


def get_nc(S=S_FULL, D=D_FULL, use_f32r=True):
    key = (S, D, use_f32r)
    if key not in _NC_CACHE:
        _NC_CACHE[key] = build_kernel(S, D, use_f32r)
    return _NC_CACHE[key]


def kernel(query, key, value, temperature):
    query = np.asarray(query, dtype=np.float32)
    key = np.asarray(key, dtype=np.float32)
    value = np.asarray(value, dtype=np.float32)
    t = np.asarray(temperature, dtype=np.float32).reshape(1, 1)
    B, S, D = query.shape

    nc = get_nc(S, D)
    in_maps = [
        {
            "query": np.ascontiguousarray(query[b]),
            "key": np.ascontiguousarray(key[b]),
            "value": np.ascontiguousarray(value[b]),
            "temperature": t,
        }
        for b in range(B)
    ]
    res = run_bass_kernel_spmd(nc, in_maps, core_ids=list(range(B)))
    attended = np.stack([res.results[b]["attended"] for b in range(B)])
    weights = np.stack([res.results[b]["weights"] for b in range(B)])
    return attended, weights


# revision 29
# speedup vs baseline: 1.2082x; 1.2082x over previous
"""Euclidean-distance attention Trainium2 Bass kernel.

Problem: B=8, Sq=Sk=2048, D=512, fp32.
  dist2[q,k]  = ||Q_q||^2 + ||K_k||^2 - 2 Q_q.K_k
  W = softmax(-dist2 / temp, axis=k);  O = W @ V
Outputs: (attended [B,Sq,D], weights [B,Sq,Sk]).

Sharding: data-parallel over batch -> 1 batch element per NeuronCore (8 cores).

Per-core algorithm (softmax is invariant to the per-row constant ||Q_q||^2,
so it is dropped; mathematically identical result):
  sim[q,k] = (2 Q.K^T - ||K||^2) / temp
  W = softmax(sim, k);  O = W @ V

Engine mapping per 128-row q tile:
  PE : QK^T in float32r (1 cy/row), W^T transposes in bf16 (1 cy/row),
       W@V in bf16 (1 cy/row)
  DVE: (S_psum - ksq_bcast) subtract, row max, PV output row scale
  ACT: exp with per-partition bias = -rowmax and fused row-sum (accum_out),
       half of the PSUM->SBUF W^T copies, prolog transpose copies
  GPS: ksq broadcast, weight normalization (1/l row scale)
"""

import numpy as np
from contextlib import ExitStack

import concourse.bass as bass
import concourse.tile as tile
from concourse import bacc, mybir
from concourse.bass import ts
from concourse.bass_utils import run_bass_kernel_spmd
from concourse.masks import make_identity

F32 = mybir.dt.float32
F32R = mybir.dt.float32r
BF16 = mybir.dt.bfloat16
AX = mybir.AxisListType
AF = mybir.ActivationFunctionType

S_FULL = 2048
D_FULL = 512
N_CORES = 8


def build_kernel(S=S_FULL, D=D_FULL, use_f32r=True, repeat=1,
                 bufs_big=3, wnorm_first=False, wt_on_dve_frac=0,
                 ablate=(), nchunk=512, s_bufs=None, t_bufs=2,
                 exp_chunk=None, wt_chunk=1024, sub_inplace=False,
                 wdma_act=False, pipeline_emit=False, wt_dma_tr=False,
                 prolog_dve=False, l_on_dve=False, pe_bcast=False,
                 ld_bufs=3, small_bufs=2):
    """Build the single-core program (one batch element)."""
    P = 128
    NT = S // P          # 128-row tiles along Sq / Sk
    DC = D // P          # 128-deep contraction chunks of d
    nchunk = min(nchunk, S)
    wt_chunk = min(wt_chunk, S)
    NN = S // nchunk     # softmax chunks of the k axis
    NMM = nchunk // 512  # matmuls per chunk (N=512 each)
    if exp_chunk is None:
        exp_chunk = nchunk
    NE = S // exp_chunk  # exp chunks
    MMDT = F32R if use_f32r else F32

    nc = bacc.Bacc("TRN2", target_bir_lowering=False, debug=False)
    q_d = nc.dram_tensor("query", [S, D], F32, kind="ExternalInput").ap()
    k_d = nc.dram_tensor("key", [S, D], F32, kind="ExternalInput").ap()
    v_d = nc.dram_tensor("value", [S, D], F32, kind="ExternalInput").ap()
    t_d = nc.dram_tensor("temperature", [1, 1], F32, kind="ExternalInput").ap()
    w_d = nc.dram_tensor("weights", [S, S], F32, kind="ExternalOutput").ap()
    o_d = nc.dram_tensor("attended", [S, D], F32, kind="ExternalOutput").ap()

    with tile.TileContext(nc) as tc, ExitStack() as ctx:
        const = ctx.enter_context(tc.tile_pool(name="const", bufs=1))
        vt_p = ctx.enter_context(tc.tile_pool(name="vt", bufs=S // P))
        kt_p = ctx.enter_context(tc.tile_pool(name="kt", bufs=S // 512))
        qt_p = ctx.enter_context(tc.tile_pool(name="qt", bufs=S // P))
        ld_p = ctx.enter_context(tc.tile_pool(name="ld", bufs=ld_bufs))
        sq_p = ctx.enter_context(tc.tile_pool(name="sq", bufs=small_bufs))
        st_p = ctx.enter_context(tc.tile_pool(name="st", bufs=bufs_big + 1))
        sim_p = ctx.enter_context(tc.tile_pool(name="sim", bufs=bufs_big))
        e_p = ctx.enter_context(tc.tile_pool(name="e", bufs=bufs_big))
        w_p = ctx.enter_context(tc.tile_pool(name="w", bufs=small_bufs))
        wt_p = ctx.enter_context(tc.tile_pool(name="wtt", bufs=bufs_big))
        o_p = ctx.enter_context(tc.tile_pool(name="o", bufs=small_bufs))
        dram_p = ctx.enter_context(tc.tile_pool(name="dr", bufs=1, space="DRAM"))
        if s_bufs is None:
            s_bufs = {512: 5, 1024: 3, 2048: 1}[nchunk]
        s_ps = ctx.enter_context(tc.tile_pool(name="sps", bufs=s_bufs, space="PSUM"))
        t_ps = ctx.enter_context(tc.tile_pool(name="tps", bufs=t_bufs, space="PSUM"))
        o_ps = ctx.enter_context(tc.tile_pool(name="ops", bufs=1, space="PSUM"))

        import contextlib
        loop_cm = tc.For_i(0, repeat, 1) if repeat > 1 else contextlib.nullcontext()
        with loop_cm:
            # ---- constants / scalars ----
            ident = const.tile([P, P], F32, tag="ident")
            make_identity(nc, ident[:])
            ident_bf = const.tile([P, P], BF16, tag="ident_bf")
            nc.vector.tensor_copy(ident_bf[:], ident[:])

            t_sb = const.tile([1, 1], F32, tag="t_sb")
            inv_t = const.tile([1, 1], F32, tag="inv_t")
            inv_t_b = const.tile([P, 1], F32, tag="inv_t_b")
            two_inv_t = const.tile([P, 1], F32, tag="two_inv_t")
            nc.sync.dma_start(t_sb[:], t_d[:])
            nc.vector.reciprocal(inv_t[:], t_sb[:])
            nc.gpsimd.partition_broadcast(inv_t_b[:], inv_t[0:1, 0:1])
            nc.scalar.mul(two_inv_t[:], inv_t_b[:], 2.0)

            # ---- K: load, ||K||^2, transpose into per-512-block KT tiles ----
            # kt_nb[nb] holds [c][k_local] for k in [nb*512,(nb+1)*512)
            kt_nb = [kt_p.tile([P, DC * 512], MMDT, tag="kt_nb", name=f"ktnb{b}") for b in range(S // 512)]
            ksq = const.tile([P, NT], F32, tag="ksq")
            for t in range(NT):
                kn = ld_p.tile([P, D], F32, tag="nat")
                nc.sync.dma_start(kn[:], k_d[ts(t, P), :])
                sq = sq_p.tile([P, D], F32, tag="sqs")
                nc.scalar.activation(
                    sq[:], kn[:], AF.Square, accum_out=ksq[:, t : t + 1]
                )
                ps = s_ps.tile([P, 512], F32, tag="s")
                for c in range(DC):
                    nc.tensor.transpose(ps[:, ts(c, P)], kn[:, ts(c, P)], ident[:])
                nb, tl = t // 4, t % 4
                out_ap = kt_nb[nb][:].rearrange("p (c k) -> p c k", k=512)[
                    :, :, ts(tl, P)
                ]
                nc.scalar.activation(
                    out_ap,
                    ps[:].rearrange("p (c j) -> p c j", j=P),
                    AF.Copy,
                    scale=two_inv_t[:, 0:1],
                )

            # ---- Q: load + transpose into per-q-tile QT tiles ----
            qt_i = []
            for i in range(NT):
                qn = ld_p.tile([P, D], F32, tag="nat")
                nc.sync.dma_start(qn[:], q_d[ts(i, P), :])
                ps = s_ps.tile([P, 512], F32, tag="s")
                for c in range(DC):
                    nc.tensor.transpose(ps[:, ts(c, P)], qn[:, ts(c, P)], ident[:])
                qt = qt_p.tile([P, D], MMDT, tag="qt_i", name=f"qti{i}")
                if prolog_dve:
                    nc.vector.tensor_copy(qt[:], ps[:])
                else:
                    nc.scalar.copy(qt[:], ps[:])
                qt_i.append(qt)

            # ---- V: per-tile bf16 (PV matmul rhs) ----
            v_j = []
            for t in range(NT):
                vn = ld_p.tile([P, D], F32, tag="nat")
                nc.sync.dma_start(vn[:], v_d[ts(t, P), :])
                vt = vt_p.tile([P, D], BF16, tag="v_j", name=f"vj{t}")
                if prolog_dve:
                    nc.vector.tensor_copy(vt[:], vn[:])
                else:
                    nc.scalar.copy(vt[:], vn[:])
                v_j.append(vt)

            # ---- ||K||^2/temp broadcast to [128, S] (full fp32) ----
            ksq_n = const.tile([P, NT], F32, tag="ksq_n")
            nc.vector.tensor_scalar_mul(ksq_n[:], ksq[:], inv_t_b[:, 0:1])
            psk = t_ps.tile([NT, P], F32, tag="tps")
            nc.tensor.transpose(psk[:], ksq_n[:], ident[:])
            sbk = const.tile([NT, P], F32, tag="ksqT")
            nc.vector.tensor_copy(sbk[:], psk[:])
            ksq_bc = const.tile([P, S], F32, tag="ksq_bc")
            scr = dram_p.tile([1, S], F32, tag="scr")
            nc.sync.dma_start(
                scr[:].rearrange("a (t p) -> (a t) p", p=P), sbk[:]
            )
            ksq_row = const.tile([1, S], F32, tag="ksq_row")
            nc.sync.dma_start(ksq_row[:], scr[:])
            if pe_bcast:
                # broadcast via K=1 outer products ones[1,128].T @ ksq_row
                # slice (short latency; avoids the slow gpsimd broadcast)
                ones_f = const.tile([1, P], F32, tag="ones_f")
                nc.gpsimd.memset(ones_f[:], 1.0)
                for g in range(S // 512):
                    psb = s_ps.tile([P, 512], F32, tag="s", name=f"psb{g}")
                    nc.tensor.matmul(
                        psb[:], ones_f[:], ksq_row[0:1, ts(g, 512)]
                    )
                    nc.vector.tensor_copy(ksq_bc[:, ts(g, 512)], psb[:])
            else:
                nc.gpsimd.partition_broadcast(ksq_bc[:], ksq_row[:])

            # ---- main loop over q tiles (emission software-pipelined:
            # tile i+1's matmuls are emitted before tile i's tail so the
            # scheduler prioritizes keeping the PE fed) ----
            def emit_head(i):
                st = {}
                st["i"] = i
                st["mx"] = st_p.tile([P, NN], F32, tag="mx", name=f"mx{i}")
                st["sim"] = sim_p.tile([P, S], F32, tag="sim", name=f"sim{i}")
                for n in range(NN):
                    ps = s_ps.tile([P, nchunk], F32, tag="s", name=f"s{i}_{n}")
                    for m in range(NMM):
                        for c in range(DC):
                            nc.tensor.matmul(
                                ps[:, ts(m, 512)],
                                qt_i[i][:, ts(c, P)],
                                kt_nb[n * NMM + m][:, ts(c, 512)],
                                start=(c == 0),
                                stop=(c == DC - 1),
                            )
                    nc.vector.tensor_sub(
                        st["sim"][:, ts(n, nchunk)], ps[:], ksq_bc[:, ts(n, nchunk)]
                    )
                    nc.vector.reduce_max(
                        st["mx"][:, n : n + 1], st["sim"][:, ts(n, nchunk)], axis=AX.X
                    )
                return st

            def emit_tail(st):
                i = st["i"]
                mx, sim_t = st["mx"], st["sim"]
                l4 = st_p.tile([P, NN], F32, tag="l4", name=f"l4{i}")
                negm = st_p.tile([P, 1], F32, tag="negm", name=f"negm{i}")
                lsum = st_p.tile([P, 1], F32, tag="lsum", name=f"lsum{i}")
                linv = st_p.tile([P, 1], F32, tag="linv", name=f"linv{i}")
                nc.vector.reduce_max(negm[:], mx[:], axis=AX.X, negate=True)

                e_t = e_p.tile([P, S], BF16, tag="e", name=f"e{i}")
                for n in range(NE):
                    if l_on_dve:
                        nc.scalar.activation(
                            e_t[:, ts(n, exp_chunk)],
                            sim_t[:, ts(n, exp_chunk)],
                            AF.Exp,
                            bias=negm[:, 0:1],
                        )
                    else:
                        nc.scalar.activation(
                            e_t[:, ts(n, exp_chunk)],
                            sim_t[:, ts(n, exp_chunk)],
                            AF.Exp,
                            bias=negm[:, 0:1],
                            accum_out=l4[:, n : n + 1],
                        )
                if l_on_dve:
                    nc.vector.reduce_sum(lsum[:], e_t[:], axis=AX.X)
                elif NE > 1:
                    nc.vector.reduce_sum(lsum[:], l4[:, 0:NE], axis=AX.X)
                else:
                    nc.vector.tensor_copy(lsum[:], l4[:, 0:1])
                nc.vector.reciprocal(linv[:], lsum[:])

                # W^T tiles in bf16 (transpose unnormalized exp)
                wt_t = wt_p.tile([P, S], BF16, tag="wt", name=f"wt{i}")
                if wt_dma_tr:
                    # hardware DMA transpose (2-byte dtype): out[p, t, j] =
                    # e[j, t*128+p], exactly the W^T block layout
                    nc.scalar.dma_start_transpose(
                        wt_t[:].rearrange("p (t j) -> p t j", j=P), e_t[:]
                    )
                else:
                    for b in range(S // wt_chunk):
                        wps = t_ps.tile([P, wt_chunk], BF16, tag="tps", name=f"tps{i}_{b}")
                        for j2 in range(wt_chunk // P):
                            j = b * (wt_chunk // P) + j2
                            nc.tensor.transpose(
                                wps[:, ts(j2, P)], e_t[:, ts(j, P)], ident_bf[:]
                            )
                        nc.scalar.copy(wt_t[:, ts(b, wt_chunk)], wps[:])

                # O = (e^T)^T @ V then row-scale by 1/l
                opsum = o_ps.tile([P, D], F32, tag="o_ps", name=f"ops{i}")
                for j in range(NT):
                    nc.tensor.matmul(
                        opsum[:],
                        wt_t[:, ts(j, P)],
                        v_j[j][:],
                        start=(j == 0),
                        stop=(j == NT - 1),
                    )
                o_t = o_p.tile([P, D], F32, tag="o", name=f"o{i}")
                nc.vector.tensor_scalar_mul(o_t[:], opsum[:], linv[:, 0:1])
                nc.sync.dma_start(o_d[ts(i, P), :], o_t[:])

                # normalized fp32 weights -> DRAM
                w_t = w_p.tile([P, S], F32, tag="w", name=f"w{i}")
                nc.vector.tensor_scalar_mul(w_t[:], e_t[:], linv[:, 0:1])
                eng = nc.scalar if wdma_act else nc.sync
                eng.dma_start(w_d[ts(i, P), :], w_t[:])

            if pipeline_emit:
                prev = emit_head(0)
                for i in range(1, NT):
                    cur = emit_head(i)
                    emit_tail(prev)
                    prev = cur
                emit_tail(prev)
            else:
                for i in range(NT):
                    emit_tail(emit_head(i))

    nc.compile()
    return nc


_NC_CACHE = {}


def get_nc(S=S_FULL, D=D_FULL, use_f32r=True):
    key = (S, D, use_f32r)
    if key not in _NC_CACHE:
        _NC_CACHE[key] = build_kernel(S, D, use_f32r)
    return _NC_CACHE[key]


def kernel(query, key, value, temperature):
    query = np.asarray(query, dtype=np.float32)
    key = np.asarray(key, dtype=np.float32)
    value = np.asarray(value, dtype=np.float32)
    t = np.asarray(temperature, dtype=np.float32).reshape(1, 1)
    B, S, D = query.shape

    nc = get_nc(S, D)
    in_maps = [
        {
            "query": np.ascontiguousarray(query[b]),
            "key": np.ascontiguousarray(key[b]),
            "value": np.ascontiguousarray(value[b]),
            "temperature": t,
        }
        for b in range(B)
    ]
    res = run_bass_kernel_spmd(nc, in_maps, core_ids=list(range(B)))
    attended = np.stack([res.results[b]["attended"] for b in range(B)])
    weights = np.stack([res.results[b]["weights"] for b in range(B)])
    return attended, weights
